# revision 2
# baseline (speedup 1.0000x reference)
"""Causal multi-head attention (B=4, N=2048, C=1024, H=16, D=64) on 8 TRN2
NeuronCores.

Sharding: core c = (batch b=c//2, head-group g=c%2). Each core projects its
batch's tokens onto its 8 heads' QKV slices, runs causal attention for those
heads, applies its slice of the output projection, and a 2-rank
ReduceScatter per batch pair sums the partial projections (chunked over 4
token ranges to overlap with compute).

Layouts (all matmuls in float32r — full-rate fp32 on the PE):
  Q^T, K^T : [feat 512, tok 2048] feature-major, produced directly by the
             projection with weights as the stationary operand.
  V        : [tok, 8*65] token-major with a ones column per head, so the
             attention-value matmul also produces softmax denominators.
  S^T      : [keys m, queries n] per 128-key block; only causal blocks are
             computed (diagonal blocks column-sliced). exp on ACT; the
             in-block triangular mask is a multiply against a constant.
  O^T      : [feat, tok], fed to the out-projection as the stationary side.
"""
import numpy as np

B, N, C, H, D = 4, 2048, 1024, 16, 64
NCORES = 8
FPC = 512          # features per core (8 heads x 64)
SCALE = D ** -0.5

_cache = {}


def _build():
    import concourse.bass as bass
    import concourse.mybir as mybir
    import concourse.tile as tile

    dt = mybir.dt
    AF = mybir.ActivationFunctionType
    f32, f32r = dt.float32, dt.float32r

    nc = bass.Bass()
    xqT = nc.declare_dram_parameter("xqT", [C, N], f32r, isOutput=False)
    xkT = nc.declare_dram_parameter("xkT", [C, N], f32r, isOutput=False)
    xvT = nc.declare_dram_parameter("xvT", [C, N], f32r, isOutput=False)
    wqT = nc.declare_dram_parameter("wqT", [C, FPC], f32r, isOutput=False)
    wkT = nc.declare_dram_parameter("wkT", [C, FPC], f32r, isOutput=False)
    wvT = nc.declare_dram_parameter("wvT", [C, FPC], f32r, isOutput=False)
    woT = nc.declare_dram_parameter("woT", [FPC, C], f32r, isOutput=False)
    bqc = nc.declare_dram_parameter("bqc", [128, 4], f32, isOutput=False)
    bkc = nc.declare_dram_parameter("bkc", [128, 4], f32, isOutput=False)
    bvr = nc.declare_dram_parameter("bvr", [1, FPC], f32r, isOutput=False)
    bor = nc.declare_dram_parameter("bor", [1, C], f32r, isOutput=False)
    tri = nc.declare_dram_parameter("tri", [128, 128], f32r, isOutput=False)
    onesr = nc.declare_dram_parameter("onesr", [1, 128], f32r, isOutput=False)
    yout = nc.declare_dram_parameter("yout", [4, 256, C], f32, isOutput=True)

    yp = nc.dram_tensor("yp", [N, C], f32)
    rsout = nc.dram_tensor("rsout", [4, 256, C], f32)
    groups = [[2 * i, 2 * i + 1] for i in range(4)]

    xq_r = xqT.rearrange("(k p) n -> p k n", p=128)
    xk_r = xkT.rearrange("(k p) n -> p k n", p=128)
    xv_r = xvT.rearrange("(k p) n -> p k n", p=128)

    with tile.TileContext(nc) as tc:
        with tc.tile_pool(name="persist", bufs=1) as pp:
            qT = pp.tile([128, 4, N], f32r)
            kT = pp.tile([128, 4, N], f32r)
            vS = pp.tile([128, 16, 8, 65], f32r)
            trS = pp.tile([128, 128], f32r)
            onS = pp.tile([1, 128], f32r)
            bqS = pp.tile([128, 4], f32)
            bkS = pp.tile([128, 4], f32)
            bvS = pp.tile([1, FPC], f32r)
            boS = pp.tile([1, C], f32r)
            nc.sync.dma_start(out=trS, in_=tri[:, :])
            nc.sync.dma_start(out=onS, in_=onesr[:, :])
            nc.sync.dma_start(out=bqS, in_=bqc[:, :])
            nc.sync.dma_start(out=bkS, in_=bkc[:, :])
            nc.sync.dma_start(out=bvS, in_=bvr[:, :])
            nc.sync.dma_start(out=boS, in_=bor[:, :])
            nc.vector.memset(vS[:, :, :, 64:65].bitcast(f32), 1.0)

            # ---------------- Phase 1: projections ----------------
            with tc.tile_pool(name="wts", bufs=1) as wp, \
                 tc.tile_pool(name="xin", bufs=2) as xp, \
                 tc.tile_pool(name="pps", bufs=4, space="PSUM") as pps:
                wq_sb = wp.tile([128, 8, FPC], f32r)
                wk_sb = wp.tile([128, 8, FPC], f32r)
                wv_sb = wp.tile([128, 8, FPC], f32r)
                nc.sync.dma_start(out=wq_sb, in_=wqT.rearrange("(k p) f -> p k f", p=128))
                nc.sync.dma_start(out=wk_sb, in_=wkT.rearrange("(k p) f -> p k f", p=128))
                nc.sync.dma_start(out=wv_sb, in_=wvT.rearrange("(k p) f -> p k f", p=128))

                # Q^T and K^T: psum[f 128, n 512] += w[:,kt,f].T @ xT[:,kt,n]
                for w_sb, x_r, dstT, bias in ((wq_sb, xq_r, qT, bqS), (wk_sb, xk_r, kT, bkS)):
                    for nt in range(4):
                        x_t = xp.tile([128, 8, 512], f32r, tag="x")
                        nc.sync.dma_start(out=x_t, in_=x_r[:, :, 512 * nt:512 * nt + 512])
                        for ft in range(4):
                            ps = pps.tile([128, 512], f32, tag="p1")
                            for kt in range(8):
                                nc.tensor.matmul(
                                    ps, w_sb[:, kt, 128 * ft:128 * ft + 128],
                                    x_t[:, kt, :], start=(kt == 0), stop=(kt == 7))
                            nc.scalar.activation(
                                dstT[:, ft, 512 * nt:512 * nt + 512], ps,
                                AF.Identity, bias=bias[:, ft:ft + 1], scale=1.0)

                # V: psum[m 128, f 512] += xvT[:,kt,m].T @ wv[:,kt,f]  (+ bias)
                for mg in range(4):
                    x_t = xp.tile([128, 8, 512], f32r, tag="x")
                    nc.sync.dma_start(out=x_t, in_=xv_r[:, :, 512 * mg:512 * mg + 512])
                    for ml in range(4):
                        mt = 4 * mg + ml
                        ps = pps.tile([128, 512], f32, tag="p1")
                        for kt in range(8):
                            nc.tensor.matmul(
                                ps, x_t[:, kt, 128 * ml:128 * ml + 128],
                                wv_sb[:, kt, :], start=(kt == 0), stop=False)
                        nc.tensor.matmul(ps, onS[0:1, :], bvS, start=False, stop=True)
                        nc.scalar.activation(
                            vS[:, mt, :, 0:64],
                            ps.rearrange("p (h e) -> p h e", h=8),
                            AF.Copy, scale=1.0)

            # ---------------- Phase 2: attention + out-proj + RS ----------------
            with tc.tile_pool(name="big2", bufs=1) as bp2, \
                 tc.tile_pool(name="ph2", bufs=3) as sp2, \
                 tc.tile_pool(name="norm", bufs=2) as np2, \
                 tc.tile_pool(name="yev", bufs=3) as yp2, \
                 tc.tile_pool(name="pss", bufs=2, space="PSUM") as pss, \
                 tc.tile_pool(name="pso", bufs=2, space="PSUM") as pso, \
                 tc.tile_pool(name="psy", bufs=2, space="PSUM") as psy:
                oT = bp2.tile([128, 4, N], f32r)
                woS = bp2.tile([128, 4, C], f32r)
                nc.sync.dma_start(out=woS, in_=woT.rearrange("(t p) c -> p t c", p=128))

                for j in range(4):
                    for h in range(8):
                        t, r0 = h // 2, (h % 2) * 64
                        po = pso.tile([128, 512], f32, tag="po")
                        nblocks = 4 * j + 4
                        first = True
                        for pair0 in range(0, nblocks, 2):
                            ps = pss.tile([128, 1024], f32, tag="s")
                            ptile = sp2.tile([128, 1024], f32r, tag="pt")
                            halves = []
                            for half in (0, 1):
                                i = pair0 + half
                                dtg = i - 4 * j
                                off = 128 * dtg if dtg >= 0 else 0
                                w = 512 - off
                                cb = 512 * half + off
                                halves.append((i, dtg, off, w, cb))
                                nc.tensor.matmul(
                                    ps[:, cb:cb + w],
                                    kT[r0:r0 + 64, t, 128 * i:128 * i + 128],
                                    qT[r0:r0 + 64, t, 512 * j + off:512 * j + 512],
                                    start=True, stop=True)
                            if halves[0][1] < 0 and halves[1][1] < 0:
                                nc.scalar.activation(ptile, ps, AF.Exp, scale=SCALE)
                            else:
                                for (i, dtg, off, w, cb) in halves:
                                    nc.scalar.activation(
                                        ptile[:, cb:cb + w], ps[:, cb:cb + w],
                                        AF.Exp, scale=SCALE)
                            for (i, dtg, off, w, cb) in halves:
                                if dtg >= 0:
                                    nc.vector.tensor_tensor(
                                        out=ptile[:, cb:cb + 128],
                                        in0=ptile[:, cb:cb + 128], in1=trS,
                                        op=mybir.AluOpType.mult)
                            for (i, dtg, off, w, cb) in halves:
                                nc.tensor.matmul(
                                    po[0:65, off:512], vS[:, i, h, :],
                                    ptile[:, cb:cb + w],
                                    start=first, stop=(i == nblocks - 1))
                                first = False
                        rec = np2.tile([1, 512], f32r, tag="rec")
                        with nc.allow_low_precision(reason="softmax denom recip in f32r"):
                            nc.vector.reciprocal(out=rec, in_=po[64:65, :])
                        pb = pss.tile([64, 512], f32, tag="s")
                        nc.tensor.matmul(pb, onS[0:1, 0:64], rec, start=True, stop=True)
                        bc = np2.tile([64, 512], f32, tag="bc")
                        nc.vector.tensor_copy(out=bc, in_=pb)
                        nc.vector.tensor_tensor(
                            out=oT[r0:r0 + 64, t, 512 * j:512 * j + 512],
                            in0=po[0:64, :], in1=bc,
                            op=mybir.AluOpType.mult)

                    # out-projection for token chunk j
                    for nb in range(4):
                        n0 = 512 * j + 128 * nb
                        for ct in range(2):
                            py = psy.tile([128, 512], f32, tag="y")
                            for ft in range(4):
                                nc.tensor.matmul(
                                    py, oT[:, ft, n0:n0 + 128],
                                    woS[:, ft, 512 * ct:512 * ct + 512],
                                    start=(ft == 0), stop=False)
                            nc.tensor.matmul(
                                py, onS[0:1, :], boS[0:1, 512 * ct:512 * ct + 512],
                                start=False, stop=True)
                            ye = yp2.tile([128, 512], f32, tag="ye")
                            nc.vector.tensor_copy(out=ye, in_=py)
                            nc.sync.dma_start(
                                out=yp[n0:n0 + 128, 512 * ct:512 * ct + 512], in_=ye)
                    nc.gpsimd.collective_compute(
                        "ReduceScatter", mybir.AluOpType.add,
                        replica_groups=groups,
                        ins=[yp[512 * j:512 * j + 512, :]],
                        outs=[rsout[j]])
                    nc.sync.dma_start(out=yout[j], in_=rsout[j])

    _legalize_waits(nc, mybir)
    return nc


def _legalize_waits(nc, mybir):
    """This walrus build accepts one sync-wait command per instruction (two
    for EventSemaphore). Hoist excess waits into standalone EventSemaphore
    instructions inserted immediately before the offender."""
    uid = 0
    for fn in nc.m.functions:
        for blk in fn.blocks:
            insts = blk.instructions
            i = 0
            while i < len(insts):
                inst = insts[i]
                si = inst.sync_info
                cap = 2 if isinstance(inst, mybir.InstEventSemaphore) else 1
                if si is not None and si.on_wait and len(si.on_wait) > cap:
                    waits = list(si.on_wait)
                    excess, keep = waits[:-cap], waits[-cap:]
                    si.on_wait = keep
                    inst.sync_info = si
                    pos = i
                    while excess:
                        chunk, excess = excess[:2], excess[2:]
                        uid += 1
                        ev = mybir.InstEventSemaphore(
                            name=f"I-waitfix-{uid}", ins=[], outs=[],
                            sync_info=mybir.SyncInfo(on_wait=chunk, on_update=[]))
                        ev.engine = inst.engine
                        insts.insert(pos, ev)
                        pos += 1
                        i += 1
                i += 1


def kernel(query, key, value, attn_mask, Wq, bq, Wk, bk, Wv, bv, Wo, bo):
    from concourse.bass_utils import run_bass_kernel_spmd

    if "nc" not in _cache:
        _cache["nc"] = _build()
    nc = _cache["nc"]

    f = np.float32
    query, key, value = (np.asarray(a, f) for a in (query, key, value))
    Wq, bq, Wk, bk = np.asarray(Wq, f), np.asarray(bq, f), np.asarray(Wk, f), np.asarray(bk, f)
    Wv, bv, Wo, bo = np.asarray(Wv, f), np.asarray(bv, f), np.asarray(Wo, f), np.asarray(bo, f)

    trin = np.triu(np.ones((128, 128), f))
    onesn = np.ones((1, 128), f)
    in_maps = []
    for c in range(NCORES):
        b, g = c // 2, c % 2
        sl = slice(g * FPC, (g + 1) * FPC)
        in_maps.append({
            "xqT": np.ascontiguousarray(query[b].T),
            "xkT": np.ascontiguousarray(key[b].T),
            "xvT": np.ascontiguousarray(value[b].T),
            "wqT": np.ascontiguousarray(Wq[sl, :].T),
            "wkT": np.ascontiguousarray(Wk[sl, :].T),
            "wvT": np.ascontiguousarray(Wv[sl, :].T),
            "woT": np.ascontiguousarray(Wo[:, sl].T),
            "bqc": np.ascontiguousarray(bq[sl].reshape(4, 128).T),
            "bkc": np.ascontiguousarray(bk[sl].reshape(4, 128).T),
            "bvr": bv[sl].reshape(1, FPC).copy(),
            "bor": (bo if g == 0 else np.zeros_like(bo)).reshape(1, C).copy(),
            "tri": trin,
            "onesr": onesn,
        })

    res = run_bass_kernel_spmd(nc, in_maps, list(range(NCORES)))
    out = np.empty((B, N, C), f)
    for b in range(B):
        y0, y1 = res.results[2 * b]["yout"], res.results[2 * b + 1]["yout"]
        for j in range(4):
            out[b, 512 * j:512 * j + 256] = y0[j]
            out[b, 512 * j + 256:512 * j + 512] = y1[j]
    return out
